# revision 28
# baseline (speedup 1.0000x reference)
"""Trainium2 Bass kernel for the Perceiver problem (nn_Perceiver_75625784148257).

Strategy (v3):
  - DEPTH=2 restarts from unchanged latents -> compute one iteration.
  - 8 cores = (batch b in 0..3) x (context half h in 0..1); flash cross-attn
    over each 25088-token half, one AllReduce per pair, latent transformer
    redundant per pair.
  - All input-derived constants are prepared on the host: per-token LayerNorm
    is applied to x there (f64), shipped pre-transposed as bf16 [32feat,
    chunk, 128tok] so the device flash loop is just score-matmul -> EXP
    (constant 1/8 scale) -> accumulate-matmul. V (including the LN bias and
    the softmax-denominator ones column) is also exact from the host.
  - bf16 operands on the PE (fp32 PSUM accumulate); latent-transformer
    weights prefetched to SBUF behind the flash loop (FF weights for the
    second block reuse the first block's buffers); all DMAs issue from the
    SP hwdge queue, keeping gpsimd free for the collective.
"""

import math
import sys

import numpy as np

sys.path.insert(0, "/opt/trn_rl_repo")

import contextlib  # noqa: E402

import ml_dtypes  # noqa: E402

import concourse.bass as bass  # noqa: E402
import concourse.mybir as mybir  # noqa: E402
from concourse.bass_utils import run_bass_kernel_spmd  # noqa: E402
from concourse.tile import TileContext  # noqa: E402

F32 = mybir.dt.float32
F32R = mybir.dt.float32r
BF16 = mybir.dt.bfloat16
AF = mybir.ActivationFunctionType
ALU = mybir.AluOpType
BF = ml_dtypes.bfloat16

# ---- problem constants ----
B, C, H, W = 4, 3, 224, 224
T_FULL = H * W            # 50176
T = T_FULL // 2           # 25088 per core
NCHUNK = T // 128         # 196
NPAIR = NCHUNK // 2       # 98
NB = 6
MAX_FREQ = 10.0
IN_DIM = 29
XF = 32
LD = 512
EPS = 1e-5
CDH = 64
LH, LDH = 8, 64
NC_CLS = 2

_CACHE = {}


def _fourier_pos():
    axes = [np.linspace(-1.0, 1.0, s) for s in (H, W)]
    grid = np.stack(np.meshgrid(*axes, indexing="ij"), axis=-1)
    x = grid[..., None]
    scales = np.linspace(1.0, MAX_FREQ / 2, NB)
    xs = x * scales * math.pi
    enc = np.concatenate([np.sin(xs), np.cos(xs), x], axis=-1)
    enc = enc.transpose(2, 3, 0, 1).reshape(-1, H, W)
    return enc.reshape(26, T_FULL)


def _split_wide_waits(nc, max_waits=1):
    for f in nc.m.functions:
        for bb in f.blocks:
            lst = bb.instructions
            i = 0
            while i < len(lst):
                inst = lst[i]
                si = inst.sync_info
                if (si is not None and si.on_wait and len(si.on_wait) > max_waits
                        and inst.engine != mybir.EngineType.Pool):
                    waits = list(si.on_wait)
                    keep = waits[-max_waits:]
                    extra = waits[:-max_waits]
                    si.on_wait = keep
                    eng = nc.engines[inst.engine]
                    new_insts = []
                    for k in range(0, len(extra), max_waits):
                        nbi = eng.nop(nofuse=True)
                        ni = nbi.ins
                        nsi = ni.sync_info
                        chunk = extra[k : k + max_waits]
                        if nsi is None:
                            ni.sync_info = mybir.SyncInfo(
                                on_wait=list(chunk), on_update=[]
                            )
                        else:
                            nsi.on_wait = list(nsi.on_wait) + list(chunk)
                        new_insts.append(ni)
                    for ni in new_insts:
                        for bb2 in f.blocks:
                            if ni in bb2.instructions:
                                bb2.instructions.remove(ni)
                                break
                    for off, ni in enumerate(new_insts):
                        lst.insert(i + off, ni)
                    i += len(new_insts) + 1
                else:
                    i += 1


# --------------------------------------------------------------------------
# kernel builder
# --------------------------------------------------------------------------
def _build():
    nc = bass.Bass()

    def P(name, shape, dt=F32):
        return nc.declare_dram_parameter(name, list(shape), dt, isOutput=False)

    t = {}
    # per-core, host-normalized transposed x and exact V (+ones col)
    t["xaT"] = P("xaT", (XF, NCHUNK, 128), BF16)
    t["v_all"] = P("v_all", (128, NCHUNK, 65), BF16)
    t["q2cT"] = P("q2cT", (XF, LD), BF16)
    # tail weights
    t["c_wo_b"] = P("c_wo_b", (CDH, LD), BF16)
    t["c_bo4"] = P("c_bo4", (128, 4))
    t["w1c"] = P("w1c", (128, 16, 4, 128), BF16)
    t["cf_b1_16"] = P("cf_b1_16", (128, 16))
    t["w2c"] = P("w2c", (128, 16, 4, 128), BF16)
    t["cf_b2_4"] = P("cf_b2_4", (128, 4))
    t["l_g4"] = P("l_g4", (128, 4))
    t["l_b4"] = P("l_b4", (128, 4))
    t["wq_l"] = P("wq_l", (128, 4, 4, 128), BF16)
    t["wk_l"] = P("wk_l", (128, 4, 4, 128), BF16)
    t["wv_l"] = P("wv_l", (128, 4, LD), BF16)
    t["wo_l"] = P("wo_l", (128, 4, 4, 128), BF16)
    t["l_bo4"] = P("l_bo4", (128, 4))
    t["w1l"] = P("w1l", (128, 16, 4, 128), BF16)
    t["lf_b1_16"] = P("lf_b1_16", (128, 16))
    t["w2l"] = P("w2l", (128, 16, 4, 128), BF16)
    t["lf_b2_4"] = P("lf_b2_4", (128, 4))
    t["h_g4"] = P("h_g4", (128, 4))
    t["h_b4"] = P("h_b4", (128, 4))
    t["h_w4"] = P("h_w4", (128, 8))
    t["h_b2"] = P("h_b2", (2, 1))
    t["e_sel"] = P("e_sel", (8, 4, 128))

    t["y_out"] = nc.declare_dram_parameter("y", [2, 1], F32, isOutput=True)

    t["o_dram"] = nc.dram_tensor("o_part", [65, 512], F32)
    t["o_red"] = nc.dram_tensor("o_red", [65, 512], F32)
    t["lrows_dram"] = nc.dram_tensor("lrows_dram", [8, 512], F32)

    with TileContext(nc) as tc:
        _body(nc, tc, t)
    _split_wide_waits(nc)
    return nc


def _body(nc, tc, t):
    t = {
        k: (v[tuple(slice(None) for _ in v.shape)]
            if type(v).__name__.endswith("TensorHandle") else v)
        for k, v in t.items()
    }
    sdma = nc.sync.dma_start

    ctx = contextlib.ExitStack()
    with ctx:
        singles = ctx.enter_context(tc.tile_pool(name="singles", bufs=1))
        wff = ctx.enter_context(tc.tile_pool(name="wff", bufs=1))

        # ---------------- flash inputs (sliced so pair 0 starts early) -----
        q2cT = singles.tile([XF, LD], BF16)
        sdma(out=q2cT, in_=t["q2cT"])
        xaT_sb = singles.tile([XF, NCHUNK, 128], BF16)
        v_sb = singles.tile([128, NCHUNK, 65], BF16)
        bounds = [0, 8, 20, 36, 56, 84, 112, 140, 168, NCHUNK]
        for s0, s1 in zip(bounds[:-1], bounds[1:]):
            sdma(out=xaT_sb[:, s0:s1, :], in_=t["xaT"][:, s0:s1, :])
            sdma(out=v_sb[:, s0:s1, :], in_=t["v_all"][:, s0:s1, :])

        # ---------------- tail weights (land during flash) ----------------
        c_wo_t = singles.tile([64, LD], BF16)
        sdma(out=c_wo_t, in_=t["c_wo_b"])
        c_bo4_t = singles.tile([128, 4], F32)
        sdma(out=c_bo4_t, in_=t["c_bo4"])
        w1c_t = wff.tile([128, 16, 4, 128], BF16, tag="fw1", name="w1c_t")
        sdma(out=w1c_t, in_=t["w1c"])
        w2c_t = wff.tile([128, 16, 4, 128], BF16, tag="fw2", name="w2c_t")
        sdma(out=w2c_t, in_=t["w2c"])
        wq_l_t = singles.tile([128, 4, 4, 128], BF16)
        sdma(out=wq_l_t, in_=t["wq_l"])
        wk_l_t = singles.tile([128, 4, 4, 128], BF16)
        sdma(out=wk_l_t, in_=t["wk_l"])
        wo_l_t = singles.tile([128, 4, 4, 128], BF16)
        sdma(out=wo_l_t, in_=t["wo_l"])
        wv_l_t = singles.tile([128, 4, LD], BF16)
        sdma(out=wv_l_t, in_=t["wv_l"])
        cf_b1_t = singles.tile([128, 16], F32)
        sdma(out=cf_b1_t, in_=t["cf_b1_16"])
        cf_b2_t = singles.tile([128, 4], F32)
        sdma(out=cf_b2_t, in_=t["cf_b2_4"])
        lf_b1_t = singles.tile([128, 16], F32)
        sdma(out=lf_b1_t, in_=t["lf_b1_16"])
        lf_b2_t = singles.tile([128, 4], F32)
        sdma(out=lf_b2_t, in_=t["lf_b2_4"])
        l_g4_t = singles.tile([128, 4], F32)
        sdma(out=l_g4_t, in_=t["l_g4"])
        l_b4_t = singles.tile([128, 4], F32)
        sdma(out=l_b4_t, in_=t["l_b4"])
        l_bo4_t = singles.tile([128, 4], F32)
        sdma(out=l_bo4_t, in_=t["l_bo4"])
        h_g4_t = singles.tile([128, 4], F32)
        sdma(out=h_g4_t, in_=t["h_g4"])
        h_b4_t = singles.tile([128, 4], F32)
        sdma(out=h_b4_t, in_=t["h_b4"])
        h_w4_t = singles.tile([128, 8], F32)
        sdma(out=h_w4_t, in_=t["h_w4"])
        h_b2_t = singles.tile([2, 1], F32)
        sdma(out=h_b2_t, in_=t["h_b2"])
        e_sel_t = singles.tile([8, 4, 128], F32)
        sdma(out=e_sel_t, in_=t["e_sel"])

        # constants
        epsc = singles.tile([128, 1], F32)
        nc.vector.memset(epsc, EPS)
        onesr64 = singles.tile([1, 64], F32)
        nc.vector.memset(onesr64, 1.0)
        onesr128 = singles.tile([1, 128], F32)
        nc.vector.memset(onesr128, 1.0)
        ones128b = singles.tile([128, 1], BF16)
        nc.vector.memset(ones128b, 1.0)
        ones128f = singles.tile([128, 1], F32)
        nc.vector.memset(ones128f, 1.0)

        o_sb = singles.tile([65, 512], F32)
        o_sum = singles.tile([65, 512], F32)

        # ================= flash =================
        fctx = contextlib.ExitStack()
        a_pool = fctx.enter_context(tc.tile_pool(name="ab", bufs=4))
        ps_s = fctx.enter_context(tc.tile_pool(name="ps_s", bufs=3, space="PSUM"))
        ps_o = fctx.enter_context(tc.tile_pool(name="ps_o", bufs=1, space="PSUM"))

        o_ps = ps_o.tile([65, 512], F32, name="o_ps")
        pend = []

        def emit_av(ent):
            a_t, p = ent
            for j in range(2):
                c = 2 * p + j
                nc.tensor.matmul(
                    o_ps, v_sb[:, c, :], a_t[:, 512 * j : 512 * (j + 1)],
                    start=(c == 0), stop=(c == NCHUNK - 1),
                )

        for p in range(NPAIR):
            sps = ps_s.tile([128, 1024], F32, tag="s_ps", name="sps")
            for j in range(2):
                c = 2 * p + j
                nc.tensor.matmul(
                    sps[:, 512 * j : 512 * (j + 1)], xaT_sb[:, c, :], q2cT,
                    start=True, stop=True,
                )
            a_t = a_pool.tile([128, 1024], BF16, tag="a_sb", name="a_sb")
            nc.scalar.activation(out=a_t, in_=sps, func=AF.Exp, scale=0.125)
            pend.append((a_t, p))
            if len(pend) >= 3:
                emit_av(pend.pop(0))
        while pend:
            emit_av(pend.pop(0))

        # drain + collective
        nc.vector.tensor_copy(o_sb, o_ps)
        sdma(out=t["o_dram"][:, :], in_=o_sb)
        nc.gpsimd.collective_compute(
            "AllReduce",
            ALU.add,
            ins=[t["o_dram"][:, :]],
            outs=[t["o_red"][:, :]],
            replica_groups=[[0, 1], [2, 3], [4, 5], [6, 7]],
        )
        sdma(out=o_sum, in_=t["o_red"][:, :])

        fctx.close()

        # ================= tail =================
        tctx = contextlib.ExitStack()
        tact = tctx.enter_context(tc.tile_pool(name="tact", bufs=1))
        h1_pool = tctx.enter_context(tc.tile_pool(name="h1p", bufs=3))
        a2_pool = tctx.enter_context(tc.tile_pool(name="a2p", bufs=2))
        ps_big = tctx.enter_context(tc.tile_pool(name="ps_big", bufs=1, space="PSUM"))
        ps_m = tctx.enter_context(tc.tile_pool(name="ps_m", bufs=2, space="PSUM"))
        with tctx:
            # o_n = o / l  -> bf16 [64, 512]   (LN bias is baked into V)
            lrow0 = tact.tile([1, 512], F32, name="lrow0")
            nc.scalar.copy(out=lrow0, in_=o_sum[64:65, :])
            l_bc = ps_m.tile([128, 512], F32, tag="m", name="l_bc0")
            nc.tensor.matmul(l_bc[0:64, :], onesr64, lrow0,
                             start=True, stop=True)
            linv_sb = tact.tile([64, 512], F32, name="linv_sb")
            nc.vector.reciprocal(linv_sb, l_bc[0:64, :])
            o_n = tact.tile([64, 512], BF16, name="o_n")
            nc.vector.tensor_mul(o_n, o_sum[0:64, :], linv_sb)

            # xT[k] = c_wo[:,k].T @ o_n + c_bo
            xT = [tact.tile([128, 512], BF16, name=f"xT{k}", tag=f"xT{k}")
                  for k in range(4)]
            for k in range(4):
                ps = ps_m.tile([128, 512], F32, tag="m", name="ps")
                nc.tensor.matmul(
                    ps, c_wo_t[:, 128 * k : 128 * (k + 1)], o_n,
                    start=True, stop=True,
                )
                nc.vector.tensor_scalar_add(xT[k], ps, c_bo4_t[:, k : k + 1])

            def ff_block(src, w1_t, b1_t, w2_t, b2_t, resid, tagp):
                x2_ps = ps_big.tile([128, 2048], F32, tag="big", name="x2_ps")

                def ff2(m, h1):
                    for k2 in range(4):
                        nc.tensor.matmul(
                            x2_ps[:, 512 * k2 : 512 * (k2 + 1)],
                            w2_t[:, m, k2, :], h1,
                            start=(m == 0), stop=(m == 15),
                        )

                pend_ff = []
                for m in range(16):
                    h_ps = ps_m.tile([128, 512], F32, tag="m", name="h_ps")
                    for k in range(4):
                        nc.tensor.matmul(
                            h_ps, w1_t[:, m, k, :], src[k],
                            start=(k == 0), stop=(k == 3),
                        )
                    h1 = h1_pool.tile([128, 512], BF16, tag="h1", name="h1")
                    nc.scalar.activation(
                        out=h1, in_=h_ps, func=AF.Gelu, bias=b1_t[:, m : m + 1]
                    )
                    pend_ff.append((m, h1))
                    if len(pend_ff) >= 2:
                        ff2(*pend_ff.pop(0))
                while pend_ff:
                    ff2(*pend_ff.pop(0))
                outs = []
                for k in range(4):
                    ot = tact.tile([128, 512], BF16, tag=f"ffo{tagp}{k}",
                                   name=f"ffo{tagp}{k}")
                    nc.vector.tensor_scalar_add(
                        ot, x2_ps[:, 512 * k : 512 * (k + 1)], b2_t[:, k : k + 1]
                    )
                    if resid is not None:
                        nc.vector.tensor_add(ot, ot, resid[k])
                    outs.append(ot)
                return outs

            x2 = ff_block(xT, w1c_t, cf_b1_t, w2c_t, cf_b2_t, xT, "c")

            # start the FF_l weight reload into the same buffers
            w1l_t = wff.tile([128, 16, 4, 128], BF16, tag="fw1", name="w1l_t")
            sdma(out=w1l_t, in_=t["w1l"])
            w2l_t = wff.tile([128, 16, 4, 128], BF16, tag="fw2", name="w2l_t")
            sdma(out=w2l_t, in_=t["w2l"])

            def ln_feat(src, g4, b4, tagp):
                s_ps = ps_m.tile([128, 512], F32, tag="m", name="lnp")
                for k in range(4):
                    nc.tensor.matmul(
                        s_ps[0:1, :], ones128b, src[k],
                        start=(k == 0), stop=(k == 3),
                    )
                s2_ps = ps_m.tile([128, 512], F32, tag="m", name="lnp2")
                for k in range(4):
                    sqt = tact.tile([128, 512], BF16, tag="lnsq",
                                    name="lnsq", bufs=2)
                    nc.vector.tensor_mul(sqt, src[k], src[k])
                    nc.tensor.matmul(
                        s2_ps[0:1, :], ones128b, sqt,
                        start=(k == 0), stop=(k == 3),
                    )
                mur = tact.tile([1, 512], F32, name=f"mur{tagp}")
                nc.vector.tensor_scalar_mul(mur, s_ps[0:1, :], 1.0 / 512.0)
                e2r = tact.tile([1, 512], F32, name=f"e2r{tagp}")
                nc.vector.tensor_scalar_mul(e2r, s2_ps[0:1, :], 1.0 / 512.0)
                musqr = tact.tile([1, 512], F32, name=f"musq{tagp}")
                nc.vector.tensor_mul(musqr, mur, mur)
                nc.vector.tensor_sub(e2r, e2r, musqr)
                sdr = tact.tile([1, 512], F32, name=f"sdr{tagp}")
                nc.scalar.activation(out=sdr, in_=e2r, func=AF.Sqrt,
                                     bias=epsc[0:1, :])
                mur_bc = ps_m.tile([128, 512], F32, tag="m", name="mur_bc")
                nc.tensor.matmul(mur_bc, onesr128, mur, start=True, stop=True)
                sd_bc = ps_m.tile([128, 512], F32, tag="m", name="sd_bc")
                nc.tensor.matmul(sd_bc, onesr128, sdr, start=True, stop=True)
                rstd_sb = tact.tile([128, 512], F32, name=f"rstd{tagp}")
                nc.vector.reciprocal(rstd_sb, sd_bc)
                outs = []
                for k in range(4):
                    ot = tact.tile([128, 512], BF16, tag=f"ln{tagp}{k}",
                                   name=f"ln{tagp}{k}")
                    nc.vector.tensor_sub(ot, src[k], mur_bc)
                    nc.vector.tensor_mul(ot, ot, rstd_sb)
                    nc.vector.tensor_scalar(
                        out=ot, in0=ot, scalar1=g4[:, k : k + 1],
                        scalar2=b4[:, k : k + 1], op0=ALU.mult, op1=ALU.add,
                    )
                    outs.append(ot)
                return outs

            xn4 = ln_feat(x2, l_g4_t, l_b4_t, "a")

            def proj4(w_t, src, tagp):
                outs = []
                for m in range(4):
                    ps = ps_m.tile([128, 512], F32, tag="m", name="pjps")
                    for k in range(4):
                        nc.tensor.matmul(
                            ps, w_t[:, m, k, :], src[k],
                            start=(k == 0), stop=(k == 3),
                        )
                    ot = tact.tile([128, 512], BF16, tag=f"pj{tagp}{m}",
                                   name=f"pj{tagp}{m}")
                    nc.scalar.copy(out=ot, in_=ps)
                    outs.append(ot)
                return outs

            qT2 = proj4(wq_l_t, xn4, "q")
            kT2 = proj4(wk_l_t, xn4, "k")

            v2_ps = ps_big.tile([128, 2048], F32, tag="big", name="v2_ps")
            for k in range(4):
                for ml in range(4):
                    nc.tensor.matmul(
                        v2_ps[:, 512 * ml : 512 * (ml + 1)],
                        xn4[k][:, 128 * ml : 128 * (ml + 1)], wv_l_t[:, k, :],
                        start=(k == 0), stop=(k == 3),
                    )
            v2 = []
            for ml in range(4):
                vt = tact.tile([128, 8, 65], BF16, tag=f"v2{ml}", name=f"v2{ml}")
                nc.vector.memset(vt, 1.0)
                nc.scalar.copy(
                    out=vt[:, :, 0:64],
                    in_=v2_ps[:, 512 * ml : 512 * (ml + 1)].rearrange(
                        "p (h d) -> p h d", h=8),
                )
                v2.append(vt)

            oT2r = [tact.tile([128, 512], BF16, tag=f"oTr{i}", name=f"oTr{i}")
                    for i in range(4)]
            lrows8 = tact.tile([8, 512], F32, name="lrows8")
            for h in range(LH):
                hq = qT2[h // 2][64 * (h % 2) : 64 * (h % 2) + 64, :]
                hk = kT2[h // 2][64 * (h % 2) : 64 * (h % 2) + 64, :]
                st_ps = ps_big.tile([128, 2048], F32, tag="big", name="st_ps")
                for s in range(4):
                    nc.tensor.matmul(
                        st_ps[:, 512 * s : 512 * (s + 1)],
                        hk[:, 128 * s : 128 * (s + 1)], hq,
                        start=True, stop=True,
                    )
                a2 = a2_pool.tile([128, 2048], BF16, tag="a2", name="a2")
                nc.scalar.activation(out=a2, in_=st_ps, func=AF.Exp, scale=0.125)
                o_ps2 = ps_m.tile([128, 512], F32, tag="m", name="o_ps2")
                for s in range(4):
                    nc.tensor.matmul(
                        o_ps2[0:65, :], v2[s][:, h, :],
                        a2[:, 512 * s : 512 * (s + 1)],
                        start=(s == 0), stop=(s == 3),
                    )
                lrow_t = tact.tile([1, 512], F32, tag="lrow_t", name="lrow_t",
                                   bufs=2)
                nc.scalar.copy(out=lrow_t, in_=o_ps2[64:65, :])
                sdma(out=t["lrows_dram"][h : h + 1, :], in_=lrow_t)
                nc.vector.tensor_copy(
                    out=oT2r[h // 2][64 * (h % 2) : 64 * (h % 2) + 64, :],
                    in_=o_ps2[0:64, :],
                )
            sdma(out=lrows8, in_=t["lrows_dram"][:, :])
            linv8 = tact.tile([8, 512], F32, name="linv8")
            nc.vector.reciprocal(linv8, lrows8)
            oT2 = [tact.tile([128, 512], BF16, tag=f"oT{i}", name=f"oT{i}")
                   for i in range(4)]
            for i in range(4):
                lsel_ps = ps_m.tile([128, 512], F32, tag="m", name="lsel_ps")
                nc.tensor.matmul(lsel_ps, e_sel_t[:, i, :], linv8,
                                 start=True, stop=True)
                nc.vector.tensor_mul(oT2[i], oT2r[i], lsel_ps)

            yT = proj4(wo_l_t, oT2, "o")
            for m in range(4):
                nc.vector.tensor_scalar_add(yT[m], yT[m], l_bo4_t[:, m : m + 1])

            zT = ff_block(yT, w1l_t, lf_b1_t, w2l_t, lf_b2_t, None, "l")

            # mean-pool over latents + final LN + head
            pool4 = tact.tile([128, 4], F32, name="pool4")
            for k in range(4):
                nc.vector.tensor_reduce(
                    pool4[:, k : k + 1], zT[k], axis=mybir.AxisListType.X,
                    op=ALU.add,
                )
            stack2 = tact.tile([128, 2], F32, name="stack2")
            nc.vector.tensor_reduce(
                stack2[:, 0:1], pool4, axis=mybir.AxisListType.X, op=ALU.add
            )
            sq4 = tact.tile([128, 4], F32, name="sq4")
            nc.vector.tensor_mul(sq4, pool4, pool4)
            nc.vector.tensor_reduce(
                stack2[:, 1:2], sq4, axis=mybir.AxisListType.X, op=ALU.add
            )
            tot_ps = ps_m.tile([128, 512], F32, tag="m", name="tot_ps")
            nc.tensor.matmul(tot_ps[0:1, 0:2], ones128f, stack2,
                             start=True, stop=True)
            tot_sb = tact.tile([1, 2], F32, name="tot_sb")
            nc.vector.tensor_copy(tot_sb, tot_ps[0:1, 0:2])
            totb_ps = ps_m.tile([128, 512], F32, tag="m", name="totb_ps")
            nc.tensor.matmul(totb_ps[:, 0:2], onesr128, tot_sb,
                             start=True, stop=True)
            muh = tact.tile([128, 1], F32, name="muh")
            nc.vector.tensor_scalar_mul(muh, totb_ps[:, 0:1], 1.0 / (512.0 * 512.0))
            e2h = tact.tile([128, 1], F32, name="e2h")
            nc.vector.tensor_scalar_mul(
                e2h, totb_ps[:, 1:2], 1.0 / (512.0 * 512.0 * 512.0)
            )
            musqh = tact.tile([128, 1], F32, name="musqh")
            nc.vector.tensor_mul(musqh, muh, muh)
            nc.vector.tensor_sub(e2h, e2h, musqh)
            sdh = tact.tile([128, 1], F32, name="sdh")
            nc.scalar.activation(out=sdh, in_=e2h, func=AF.Sqrt, bias=epsc)
            rstdh = tact.tile([128, 1], F32, name="rstdh")
            nc.vector.reciprocal(rstdh, sdh)
            pn4 = tact.tile([128, 4], F32, name="pn4")
            nc.vector.tensor_scalar(
                out=pn4, in0=pool4, scalar1=1.0 / 512.0, scalar2=muh,
                op0=ALU.mult, op1=ALU.subtract,
            )
            nc.vector.tensor_scalar_mul(pn4, pn4, rstdh)
            nc.vector.tensor_mul(pn4, pn4, h_g4_t)
            nc.vector.tensor_add(pn4, pn4, h_b4_t)
            y_ps = ps_m.tile([128, 512], F32, tag="m", name="y_ps")
            for k in range(4):
                nc.tensor.matmul(
                    y_ps[0:2, 0:1], h_w4_t[:, 2 * k : 2 * k + 2],
                    pn4[:, k : k + 1],
                    start=(k == 0), stop=(k == 3),
                )
            yo = tact.tile([2, 1], F32, name="yo")
            nc.vector.tensor_add(yo, y_ps[0:2, 0:1], h_b2_t)
            sdma(out=t["y_out"][:, :], in_=yo)


# --------------------------------------------------------------------------
# host glue
# --------------------------------------------------------------------------
def _col4(v):
    return np.ascontiguousarray(v.reshape(4, 128).T.astype(np.float32))


def _e_sel():
    e = np.zeros((8, 4, 128), dtype=np.float32)
    for i in range(4):
        e[2 * i, i, 0:64] = 1.0
        e[2 * i + 1, i, 64:128] = 1.0
    return np.ascontiguousarray(e)


def _host_flash_weights(I):
    """q2cT (centered+gained, f64) -> bf16 [32, 512]."""
    lat = I["latents"].astype(np.float64)
    mu = lat.mean(-1, keepdims=True)
    var = ((lat - mu) ** 2).mean(-1, keepdims=True)
    xq = (lat - mu) / np.sqrt(var + EPS) * I["c_ln_g"] + I["c_ln_b"]
    q = xq @ I["c_wq"].astype(np.float64)          # [512, 64]
    q2 = I["c_wk"].astype(np.float64) @ q.T        # [29, 512]
    q2g = q2 * I["ctx_ln_g"].astype(np.float64)[:, None]
    q2c = q2g - q2g.mean(0, keepdims=True)
    q2cT = np.zeros((XF, LD), dtype=BF)
    q2cT[0:29, :] = q2c.astype(BF)
    return np.ascontiguousarray(q2cT)


def _prep_maps(inputs):
    I = {k: np.asarray(v, np.float32) for k, v in inputs.items()}
    enc = _fourier_pos()  # (26, T_FULL) f64
    q2cT = _host_flash_weights(I)

    def ffw1(w):
        return np.ascontiguousarray(
            w.reshape(4, 128, 16, 128).transpose(1, 2, 0, 3).astype(BF))

    def ffw2(w):
        return np.ascontiguousarray(
            w.reshape(16, 128, 4, 128).transpose(1, 0, 2, 3).astype(BF))

    def proj_w(w):
        return np.ascontiguousarray(
            w.reshape(4, 128, 4, 128).transpose(1, 2, 0, 3).astype(BF))

    shared = {
        "q2cT": q2cT,
        "c_wo_b": I["c_wo"].astype(BF),
        "c_bo4": _col4(I["c_bo"]),
        "w1c": ffw1(I["cf_w1"]),
        "cf_b1_16": np.ascontiguousarray(I["cf_b1"].reshape(16, 128).T),
        "w2c": ffw2(I["cf_w2"]),
        "cf_b2_4": _col4(I["cf_b2"]),
        "l_g4": _col4(I["l_ln_g"]),
        "l_b4": _col4(I["l_ln_b"]),
        "wq_l": proj_w(I["l_wq"]),
        "wk_l": proj_w(I["l_wk"]),
        "wv_l": np.ascontiguousarray(
            I["l_wv"].reshape(4, 128, 512).transpose(1, 0, 2).astype(BF)),
        "wo_l": proj_w(I["l_wo"]),
        "l_bo4": _col4(I["l_bo"]),
        "w1l": ffw1(I["lf_w1"]),
        "lf_b1_16": np.ascontiguousarray(I["lf_b1"].reshape(16, 128).T),
        "w2l": ffw2(I["lf_w2"]),
        "lf_b2_4": _col4(I["lf_b2"]),
        "h_g4": _col4(I["h_ln_g"]),
        "h_b4": _col4(I["h_ln_b"]),
        "h_w4": np.ascontiguousarray(
            I["h_w"].reshape(4, 128, 2).transpose(1, 0, 2).reshape(128, 8)
        ),
        "h_b2": I["h_b"][:, None],
        "e_sel": _e_sel(),
    }
    shared = {
        k: (np.ascontiguousarray(v, dtype=np.float32)
            if v.dtype != BF else v)
        for k, v in shared.items()
    }

    ctx_g = I["ctx_ln_g"].astype(np.float64)
    ctx_b = I["ctx_ln_b"].astype(np.float64)
    c_wv = I["c_wv"].astype(np.float64)

    maps = []
    for core in range(8):
        b, h = core // 2, core % 2
        data = I["data"][b].reshape(3, T_FULL)[:, h * T : (h + 1) * T]
        x = np.concatenate([data.astype(np.float64),
                            enc[:, h * T : (h + 1) * T]], 0)  # [29, T] f64
        mu = x.mean(0, keepdims=True)
        sd = np.sqrt(((x - mu) ** 2).mean(0, keepdims=True) + EPS)
        xn = (x - mu) / sd                                    # [29, T]
        xaT = np.zeros((XF, NCHUNK, 128), dtype=BF)
        xaT[0:29] = xn.reshape(29, NCHUNK, 128).astype(BF)
        # exact V with LN gain+bias, plus softmax-denominator ones column
        ln_x = xn * ctx_g[:, None] + ctx_b[:, None]
        v = (ln_x.T @ c_wv)                                   # [T, 64]
        v_all = np.ones((128, NCHUNK, 65), dtype=BF)
        v_all[:, :, 0:64] = v.reshape(NCHUNK, 128, 64).transpose(1, 0, 2).astype(BF)
        m = dict(shared)
        m["xaT"] = np.ascontiguousarray(xaT)
        m["v_all"] = np.ascontiguousarray(v_all)
        maps.append(m)
    return maps


def _get_nc():
    if "nc" not in _CACHE:
        _CACHE["nc"] = _build()
    return _CACHE["nc"]


def run_cores(inputs, **kw):
    nc = _get_nc()
    maps = _prep_maps(inputs)
    return run_bass_kernel_spmd(nc, maps, list(range(8)), **kw)


def kernel(**inputs) -> np.ndarray:
    res = run_cores(inputs)
    out = np.zeros((4, NC_CLS), np.float32)
    for b in range(4):
        out[b] = res.results[2 * b]["y"][:, 0]
    return out


# revision 29
# speedup vs baseline: 1.1234x; 1.1234x over previous
"""Trainium2 Bass kernel for the Perceiver problem (nn_Perceiver_75625784148257).

Strategy (v3):
  - DEPTH=2 restarts from unchanged latents -> compute one iteration.
  - 8 cores = (batch b in 0..3) x (context half h in 0..1); flash cross-attn
    over each 25088-token half, one AllReduce per pair, latent transformer
    redundant per pair.
  - All input-derived constants are prepared on the host: per-token LayerNorm
    is applied to x there (f64), shipped pre-transposed as bf16 [32feat,
    chunk, 128tok] so the device flash loop is just score-matmul -> EXP
    (constant 1/8 scale) -> accumulate-matmul. V (including the LN bias and
    the softmax-denominator ones column) is also exact from the host.
  - bf16 operands on the PE (fp32 PSUM accumulate); latent-transformer
    weights prefetched to SBUF behind the flash loop (FF weights for the
    second block reuse the first block's buffers); all DMAs issue from the
    SP hwdge queue, keeping gpsimd free for the collective.
"""

import math
import sys

import numpy as np

sys.path.insert(0, "/opt/trn_rl_repo")

import contextlib  # noqa: E402

import ml_dtypes  # noqa: E402

import concourse.bass as bass  # noqa: E402
import concourse.mybir as mybir  # noqa: E402
from concourse.bass_utils import run_bass_kernel_spmd  # noqa: E402
from concourse.tile import TileContext  # noqa: E402

F32 = mybir.dt.float32
F32R = mybir.dt.float32r
BF16 = mybir.dt.bfloat16
AF = mybir.ActivationFunctionType
ALU = mybir.AluOpType
BF = ml_dtypes.bfloat16

# ---- problem constants ----
B, C, H, W = 4, 3, 224, 224
T_FULL = H * W            # 50176
T = T_FULL // 2           # 25088 per core
NCHUNK = T // 128         # 196
NPAIR = NCHUNK // 2       # 98
NB = 6
MAX_FREQ = 10.0
IN_DIM = 29
XF = 32
LD = 512
EPS = 1e-5
CDH = 64
LH, LDH = 8, 64
NC_CLS = 2

_CACHE = {}


def _fourier_pos():
    axes = [np.linspace(-1.0, 1.0, s) for s in (H, W)]
    grid = np.stack(np.meshgrid(*axes, indexing="ij"), axis=-1)
    x = grid[..., None]
    scales = np.linspace(1.0, MAX_FREQ / 2, NB)
    xs = x * scales * math.pi
    enc = np.concatenate([np.sin(xs), np.cos(xs), x], axis=-1)
    enc = enc.transpose(2, 3, 0, 1).reshape(-1, H, W)
    return enc.reshape(26, T_FULL)


def _split_wide_waits(nc, max_waits=1):
    for f in nc.m.functions:
        for bb in f.blocks:
            lst = bb.instructions
            i = 0
            while i < len(lst):
                inst = lst[i]
                si = inst.sync_info
                if (si is not None and si.on_wait and len(si.on_wait) > max_waits
                        and inst.engine != mybir.EngineType.Pool):
                    waits = list(si.on_wait)
                    keep = waits[-max_waits:]
                    extra = waits[:-max_waits]
                    si.on_wait = keep
                    eng = nc.engines[inst.engine]
                    new_insts = []
                    for k in range(0, len(extra), max_waits):
                        nbi = eng.nop(nofuse=True)
                        ni = nbi.ins
                        nsi = ni.sync_info
                        chunk = extra[k : k + max_waits]
                        if nsi is None:
                            ni.sync_info = mybir.SyncInfo(
                                on_wait=list(chunk), on_update=[]
                            )
                        else:
                            nsi.on_wait = list(nsi.on_wait) + list(chunk)
                        new_insts.append(ni)
                    for ni in new_insts:
                        for bb2 in f.blocks:
                            if ni in bb2.instructions:
                                bb2.instructions.remove(ni)
                                break
                    for off, ni in enumerate(new_insts):
                        lst.insert(i + off, ni)
                    i += len(new_insts) + 1
                else:
                    i += 1


# --------------------------------------------------------------------------
# kernel builder
# --------------------------------------------------------------------------
def _build():
    nc = bass.Bass()

    def P(name, shape, dt=F32):
        return nc.declare_dram_parameter(name, list(shape), dt, isOutput=False)

    t = {}
    # per-core, host-normalized transposed x and exact V (+ones col)
    t["xaT"] = P("xaT", (2 * XF, NPAIR, 128), BF16)
    t["v_all"] = P("v_all", (128, NCHUNK, 65), BF16)
    t["q2cT"] = P("q2cT", (2 * XF, LD), BF16)
    # tail weights
    t["c_wo_b"] = P("c_wo_b", (CDH, LD), BF16)
    t["c_bo4"] = P("c_bo4", (128, 4))
    t["w1c"] = P("w1c", (128, 16, 4, 128), BF16)
    t["cf_b1_16"] = P("cf_b1_16", (128, 16))
    t["w2c"] = P("w2c", (128, 16, 4, 128), BF16)
    t["cf_b2_4"] = P("cf_b2_4", (128, 4))
    t["l_g4"] = P("l_g4", (128, 4))
    t["l_b4"] = P("l_b4", (128, 4))
    t["wq_l"] = P("wq_l", (128, 4, 4, 128), BF16)
    t["wk_l"] = P("wk_l", (128, 4, 4, 128), BF16)
    t["wv_l"] = P("wv_l", (128, 4, LD), BF16)
    t["wo_l"] = P("wo_l", (128, 4, 4, 128), BF16)
    t["l_bo4"] = P("l_bo4", (128, 4))
    t["w1l"] = P("w1l", (128, 16, 4, 128), BF16)
    t["lf_b1_16"] = P("lf_b1_16", (128, 16))
    t["w2l"] = P("w2l", (128, 16, 4, 128), BF16)
    t["lf_b2_4"] = P("lf_b2_4", (128, 4))
    t["h_g4"] = P("h_g4", (128, 4))
    t["h_b4"] = P("h_b4", (128, 4))
    t["h_w4"] = P("h_w4", (128, 8))
    t["h_b2"] = P("h_b2", (2, 1))
    t["e_sel"] = P("e_sel", (8, 4, 128))

    t["y_out"] = nc.declare_dram_parameter("y", [2, 1], F32, isOutput=True)

    t["o_dram"] = nc.dram_tensor("o_part", [65, 512], F32)
    t["o_red"] = nc.dram_tensor("o_red", [65, 512], F32)
    t["lrows_dram"] = nc.dram_tensor("lrows_dram", [8, 512], F32)

    with TileContext(nc) as tc:
        _body(nc, tc, t)
    _split_wide_waits(nc)
    return nc


def _body(nc, tc, t):
    t = {
        k: (v[tuple(slice(None) for _ in v.shape)]
            if type(v).__name__.endswith("TensorHandle") else v)
        for k, v in t.items()
    }
    sdma = nc.sync.dma_start

    ctx = contextlib.ExitStack()
    with ctx:
        singles = ctx.enter_context(tc.tile_pool(name="singles", bufs=1))
        wff = ctx.enter_context(tc.tile_pool(name="wff", bufs=1))

        # ---------------- flash inputs (sliced so pair 0 starts early) -----
        q2cT = singles.tile([2 * XF, LD], BF16)
        sdma(out=q2cT, in_=t["q2cT"])
        xp = []
        for p in range(NPAIR):
            xt = singles.tile([2 * XF, 128], BF16, tag=f"xp{p}", name=f"xp{p}")
            xp.append(xt)
        v_sb = singles.tile([128, NCHUNK, 65], BF16)
        vbounds = [0, 8, 24, 48, 80, 120, 160, NCHUNK]
        vi = 0
        for p in range(NPAIR):
            sdma(out=xp[p], in_=t["xaT"][:, p, :])
            if vi < len(vbounds) - 1 and p >= vbounds[vi + 1] // 2:
                s0, s1 = vbounds[vi], vbounds[vi + 1]
                sdma(out=v_sb[:, s0:s1, :], in_=t["v_all"][:, s0:s1, :])
                vi += 1
        while vi < len(vbounds) - 1:
            s0, s1 = vbounds[vi], vbounds[vi + 1]
            sdma(out=v_sb[:, s0:s1, :], in_=t["v_all"][:, s0:s1, :])
            vi += 1

        # ---------------- tail weights (land during flash) ----------------
        c_wo_t = singles.tile([64, LD], BF16)
        sdma(out=c_wo_t, in_=t["c_wo_b"])
        c_bo4_t = singles.tile([128, 4], F32)
        sdma(out=c_bo4_t, in_=t["c_bo4"])
        w1c_t = wff.tile([128, 16, 4, 128], BF16, tag="fw1", name="w1c_t")
        sdma(out=w1c_t, in_=t["w1c"])
        w2c_t = wff.tile([128, 16, 4, 128], BF16, tag="fw2", name="w2c_t")
        sdma(out=w2c_t, in_=t["w2c"])
        wq_l_t = singles.tile([128, 4, 4, 128], BF16)
        sdma(out=wq_l_t, in_=t["wq_l"])
        wk_l_t = singles.tile([128, 4, 4, 128], BF16)
        sdma(out=wk_l_t, in_=t["wk_l"])
        wo_l_t = singles.tile([128, 4, 4, 128], BF16)
        sdma(out=wo_l_t, in_=t["wo_l"])
        wv_l_t = singles.tile([128, 4, LD], BF16)
        sdma(out=wv_l_t, in_=t["wv_l"])
        cf_b1_t = singles.tile([128, 16], F32)
        sdma(out=cf_b1_t, in_=t["cf_b1_16"])
        cf_b2_t = singles.tile([128, 4], F32)
        sdma(out=cf_b2_t, in_=t["cf_b2_4"])
        lf_b1_t = singles.tile([128, 16], F32)
        sdma(out=lf_b1_t, in_=t["lf_b1_16"])
        lf_b2_t = singles.tile([128, 4], F32)
        sdma(out=lf_b2_t, in_=t["lf_b2_4"])
        l_g4_t = singles.tile([128, 4], F32)
        sdma(out=l_g4_t, in_=t["l_g4"])
        l_b4_t = singles.tile([128, 4], F32)
        sdma(out=l_b4_t, in_=t["l_b4"])
        l_bo4_t = singles.tile([128, 4], F32)
        sdma(out=l_bo4_t, in_=t["l_bo4"])
        h_g4_t = singles.tile([128, 4], F32)
        sdma(out=h_g4_t, in_=t["h_g4"])
        h_b4_t = singles.tile([128, 4], F32)
        sdma(out=h_b4_t, in_=t["h_b4"])
        h_w4_t = singles.tile([128, 8], F32)
        sdma(out=h_w4_t, in_=t["h_w4"])
        h_b2_t = singles.tile([2, 1], F32)
        sdma(out=h_b2_t, in_=t["h_b2"])
        e_sel_t = singles.tile([8, 4, 128], F32)
        sdma(out=e_sel_t, in_=t["e_sel"])

        # constants
        epsc = singles.tile([128, 1], F32)
        nc.vector.memset(epsc, EPS)
        onesr64 = singles.tile([1, 64], F32)
        nc.vector.memset(onesr64, 1.0)
        onesr128 = singles.tile([1, 128], F32)
        nc.vector.memset(onesr128, 1.0)
        ones128b = singles.tile([128, 1], BF16)
        nc.vector.memset(ones128b, 1.0)
        ones128f = singles.tile([128, 1], F32)
        nc.vector.memset(ones128f, 1.0)

        o_sb = singles.tile([65, 512], F32)
        o_sum = singles.tile([65, 512], F32)

        # ================= flash =================
        fctx = contextlib.ExitStack()
        a_pool = fctx.enter_context(tc.tile_pool(name="ab", bufs=4))
        ps_s = fctx.enter_context(tc.tile_pool(name="ps_s", bufs=3, space="PSUM"))
        ps_o = fctx.enter_context(tc.tile_pool(name="ps_o", bufs=1, space="PSUM"))

        o_ps = ps_o.tile([65, 512], F32, name="o_ps")
        pend = []

        def emit_av(ent):
            a_t, p = ent
            for j in range(2):
                c = 2 * p + j
                nc.tensor.matmul(
                    o_ps, v_sb[:, c, :], a_t[:, 512 * j : 512 * (j + 1)],
                    start=(c == 0), stop=(c == NCHUNK - 1),
                )

        for p in range(NPAIR):
            sps = ps_s.tile([128, 1024], F32, tag="s_ps", name="sps")
            for j in range(2):
                nc.tensor.matmul(
                    sps[:, 512 * j : 512 * (j + 1)],
                    xp[p][32 * j : 32 * (j + 1), :],
                    q2cT[32 * j : 32 * (j + 1), :],
                    start=True, stop=True,
                )
            a_t = a_pool.tile([128, 1024], BF16, tag="a_sb", name="a_sb")
            nc.scalar.activation(out=a_t, in_=sps, func=AF.Exp, scale=0.125)
            pend.append((a_t, p))
            if len(pend) >= 3:
                emit_av(pend.pop(0))
        while pend:
            emit_av(pend.pop(0))

        # drain + collective
        nc.vector.tensor_copy(o_sb, o_ps)
        sdma(out=t["o_dram"][:, :], in_=o_sb)
        nc.gpsimd.collective_compute(
            "AllReduce",
            ALU.add,
            ins=[t["o_dram"][:, :]],
            outs=[t["o_red"][:, :]],
            replica_groups=[[0, 1], [2, 3], [4, 5], [6, 7]],
        )
        sdma(out=o_sum, in_=t["o_red"][:, :])

        fctx.close()

        # ================= tail =================
        tctx = contextlib.ExitStack()
        tact = tctx.enter_context(tc.tile_pool(name="tact", bufs=1))
        h1_pool = tctx.enter_context(tc.tile_pool(name="h1p", bufs=3))
        a2_pool = tctx.enter_context(tc.tile_pool(name="a2p", bufs=2))
        ps_big = tctx.enter_context(tc.tile_pool(name="ps_big", bufs=1, space="PSUM"))
        ps_m = tctx.enter_context(tc.tile_pool(name="ps_m", bufs=2, space="PSUM"))
        with tctx:
            # o_n = o / l  -> bf16 [64, 512]   (LN bias is baked into V)
            lrow0 = tact.tile([1, 512], F32, name="lrow0")
            nc.scalar.copy(out=lrow0, in_=o_sum[64:65, :])
            l_bc = ps_m.tile([128, 512], F32, tag="m", name="l_bc0")
            nc.tensor.matmul(l_bc[0:64, :], onesr64, lrow0,
                             start=True, stop=True)
            linv_sb = tact.tile([64, 512], F32, name="linv_sb")
            nc.vector.reciprocal(linv_sb, l_bc[0:64, :])
            o_n = tact.tile([64, 512], BF16, name="o_n")
            nc.vector.tensor_mul(o_n, o_sum[0:64, :], linv_sb)

            # xT[k] = c_wo[:,k].T @ o_n + c_bo
            xT = [tact.tile([128, 512], BF16, name=f"xT{k}", tag=f"xT{k}")
                  for k in range(4)]
            for k in range(4):
                ps = ps_m.tile([128, 512], F32, tag="m", name="ps")
                nc.tensor.matmul(
                    ps, c_wo_t[:, 128 * k : 128 * (k + 1)], o_n,
                    start=True, stop=True,
                )
                nc.vector.tensor_scalar_add(xT[k], ps, c_bo4_t[:, k : k + 1])

            def ff_block(src, w1_t, b1_t, w2_t, b2_t, resid, tagp):
                x2_ps = ps_big.tile([128, 2048], F32, tag="big", name="x2_ps")

                def ff2(m, h1):
                    for k2 in range(4):
                        nc.tensor.matmul(
                            x2_ps[:, 512 * k2 : 512 * (k2 + 1)],
                            w2_t[:, m, k2, :], h1,
                            start=(m == 0), stop=(m == 15),
                        )

                pend_ff = []
                for m in range(16):
                    h_ps = ps_m.tile([128, 512], F32, tag="m", name="h_ps")
                    for k in range(4):
                        nc.tensor.matmul(
                            h_ps, w1_t[:, m, k, :], src[k],
                            start=(k == 0), stop=(k == 3),
                        )
                    h1 = h1_pool.tile([128, 512], BF16, tag="h1", name="h1")
                    nc.scalar.activation(
                        out=h1, in_=h_ps, func=AF.Gelu, bias=b1_t[:, m : m + 1]
                    )
                    pend_ff.append((m, h1))
                    if len(pend_ff) >= 2:
                        ff2(*pend_ff.pop(0))
                while pend_ff:
                    ff2(*pend_ff.pop(0))
                outs = []
                for k in range(4):
                    ot = tact.tile([128, 512], BF16, tag=f"ffo{tagp}{k}",
                                   name=f"ffo{tagp}{k}")
                    nc.vector.tensor_scalar_add(
                        ot, x2_ps[:, 512 * k : 512 * (k + 1)], b2_t[:, k : k + 1]
                    )
                    if resid is not None:
                        nc.vector.tensor_add(ot, ot, resid[k])
                    outs.append(ot)
                return outs

            x2 = ff_block(xT, w1c_t, cf_b1_t, w2c_t, cf_b2_t, xT, "c")

            # start the FF_l weight reload into the same buffers
            w1l_t = wff.tile([128, 16, 4, 128], BF16, tag="fw1", name="w1l_t")
            sdma(out=w1l_t, in_=t["w1l"])
            w2l_t = wff.tile([128, 16, 4, 128], BF16, tag="fw2", name="w2l_t")
            sdma(out=w2l_t, in_=t["w2l"])

            def ln_feat(src, g4, b4, tagp):
                s_ps = ps_m.tile([128, 512], F32, tag="m", name="lnp")
                for k in range(4):
                    nc.tensor.matmul(
                        s_ps[0:1, :], ones128b, src[k],
                        start=(k == 0), stop=(k == 3),
                    )
                s2_ps = ps_m.tile([128, 512], F32, tag="m", name="lnp2")
                for k in range(4):
                    sqt = tact.tile([128, 512], BF16, tag="lnsq",
                                    name="lnsq", bufs=2)
                    nc.vector.tensor_mul(sqt, src[k], src[k])
                    nc.tensor.matmul(
                        s2_ps[0:1, :], ones128b, sqt,
                        start=(k == 0), stop=(k == 3),
                    )
                mur = tact.tile([1, 512], F32, name=f"mur{tagp}")
                nc.vector.tensor_scalar_mul(mur, s_ps[0:1, :], 1.0 / 512.0)
                e2r = tact.tile([1, 512], F32, name=f"e2r{tagp}")
                nc.vector.tensor_scalar_mul(e2r, s2_ps[0:1, :], 1.0 / 512.0)
                musqr = tact.tile([1, 512], F32, name=f"musq{tagp}")
                nc.vector.tensor_mul(musqr, mur, mur)
                nc.vector.tensor_sub(e2r, e2r, musqr)
                sdr = tact.tile([1, 512], F32, name=f"sdr{tagp}")
                nc.scalar.activation(out=sdr, in_=e2r, func=AF.Sqrt,
                                     bias=epsc[0:1, :])
                mur_bc = ps_m.tile([128, 512], F32, tag="m", name="mur_bc")
                nc.tensor.matmul(mur_bc, onesr128, mur, start=True, stop=True)
                sd_bc = ps_m.tile([128, 512], F32, tag="m", name="sd_bc")
                nc.tensor.matmul(sd_bc, onesr128, sdr, start=True, stop=True)
                rstd_sb = tact.tile([128, 512], F32, name=f"rstd{tagp}")
                nc.vector.reciprocal(rstd_sb, sd_bc)
                outs = []
                for k in range(4):
                    ot = tact.tile([128, 512], BF16, tag=f"ln{tagp}{k}",
                                   name=f"ln{tagp}{k}")
                    nc.vector.tensor_sub(ot, src[k], mur_bc)
                    nc.vector.tensor_mul(ot, ot, rstd_sb)
                    nc.vector.tensor_scalar(
                        out=ot, in0=ot, scalar1=g4[:, k : k + 1],
                        scalar2=b4[:, k : k + 1], op0=ALU.mult, op1=ALU.add,
                    )
                    outs.append(ot)
                return outs

            xn4 = ln_feat(x2, l_g4_t, l_b4_t, "a")

            def proj4(w_t, src, tagp):
                outs = []
                for m in range(4):
                    ps = ps_m.tile([128, 512], F32, tag="m", name="pjps")
                    for k in range(4):
                        nc.tensor.matmul(
                            ps, w_t[:, m, k, :], src[k],
                            start=(k == 0), stop=(k == 3),
                        )
                    ot = tact.tile([128, 512], BF16, tag=f"pj{tagp}{m}",
                                   name=f"pj{tagp}{m}")
                    nc.scalar.copy(out=ot, in_=ps)
                    outs.append(ot)
                return outs

            qT2 = proj4(wq_l_t, xn4, "q")
            kT2 = proj4(wk_l_t, xn4, "k")

            v2_ps = ps_big.tile([128, 2048], F32, tag="big", name="v2_ps")
            for k in range(4):
                for ml in range(4):
                    nc.tensor.matmul(
                        v2_ps[:, 512 * ml : 512 * (ml + 1)],
                        xn4[k][:, 128 * ml : 128 * (ml + 1)], wv_l_t[:, k, :],
                        start=(k == 0), stop=(k == 3),
                    )
            v2 = []
            for ml in range(4):
                vt = tact.tile([128, 8, 65], BF16, tag=f"v2{ml}", name=f"v2{ml}")
                nc.vector.memset(vt, 1.0)
                nc.scalar.copy(
                    out=vt[:, :, 0:64],
                    in_=v2_ps[:, 512 * ml : 512 * (ml + 1)].rearrange(
                        "p (h d) -> p h d", h=8),
                )
                v2.append(vt)

            oT2r = [tact.tile([128, 512], BF16, tag=f"oTr{i}", name=f"oTr{i}")
                    for i in range(4)]
            lrows8 = tact.tile([8, 512], F32, name="lrows8")
            for h in range(LH):
                hq = qT2[h // 2][64 * (h % 2) : 64 * (h % 2) + 64, :]
                hk = kT2[h // 2][64 * (h % 2) : 64 * (h % 2) + 64, :]
                st_ps = ps_big.tile([128, 2048], F32, tag="big", name="st_ps")
                for s in range(4):
                    nc.tensor.matmul(
                        st_ps[:, 512 * s : 512 * (s + 1)],
                        hk[:, 128 * s : 128 * (s + 1)], hq,
                        start=True, stop=True,
                    )
                a2 = a2_pool.tile([128, 2048], BF16, tag="a2", name="a2")
                nc.scalar.activation(out=a2, in_=st_ps, func=AF.Exp, scale=0.125)
                o_ps2 = ps_m.tile([128, 512], F32, tag="m", name="o_ps2")
                for s in range(4):
                    nc.tensor.matmul(
                        o_ps2[0:65, :], v2[s][:, h, :],
                        a2[:, 512 * s : 512 * (s + 1)],
                        start=(s == 0), stop=(s == 3),
                    )
                lrow_t = tact.tile([1, 512], F32, tag="lrow_t", name="lrow_t",
                                   bufs=2)
                nc.scalar.copy(out=lrow_t, in_=o_ps2[64:65, :])
                sdma(out=t["lrows_dram"][h : h + 1, :], in_=lrow_t)
                nc.vector.tensor_copy(
                    out=oT2r[h // 2][64 * (h % 2) : 64 * (h % 2) + 64, :],
                    in_=o_ps2[0:64, :],
                )
            sdma(out=lrows8, in_=t["lrows_dram"][:, :])
            linv8 = tact.tile([8, 512], F32, name="linv8")
            nc.vector.reciprocal(linv8, lrows8)
            oT2 = [tact.tile([128, 512], BF16, tag=f"oT{i}", name=f"oT{i}")
                   for i in range(4)]
            for i in range(4):
                lsel_ps = ps_m.tile([128, 512], F32, tag="m", name="lsel_ps")
                nc.tensor.matmul(lsel_ps, e_sel_t[:, i, :], linv8,
                                 start=True, stop=True)
                nc.vector.tensor_mul(oT2[i], oT2r[i], lsel_ps)

            yT = proj4(wo_l_t, oT2, "o")
            for m in range(4):
                nc.vector.tensor_scalar_add(yT[m], yT[m], l_bo4_t[:, m : m + 1])

            zT = ff_block(yT, w1l_t, lf_b1_t, w2l_t, lf_b2_t, None, "l")

            # mean-pool over latents + final LN + head
            pool4 = tact.tile([128, 4], F32, name="pool4")
            for k in range(4):
                nc.vector.tensor_reduce(
                    pool4[:, k : k + 1], zT[k], axis=mybir.AxisListType.X,
                    op=ALU.add,
                )
            stack2 = tact.tile([128, 2], F32, name="stack2")
            nc.vector.tensor_reduce(
                stack2[:, 0:1], pool4, axis=mybir.AxisListType.X, op=ALU.add
            )
            sq4 = tact.tile([128, 4], F32, name="sq4")
            nc.vector.tensor_mul(sq4, pool4, pool4)
            nc.vector.tensor_reduce(
                stack2[:, 1:2], sq4, axis=mybir.AxisListType.X, op=ALU.add
            )
            tot_ps = ps_m.tile([128, 512], F32, tag="m", name="tot_ps")
            nc.tensor.matmul(tot_ps[0:1, 0:2], ones128f, stack2,
                             start=True, stop=True)
            tot_sb = tact.tile([1, 2], F32, name="tot_sb")
            nc.vector.tensor_copy(tot_sb, tot_ps[0:1, 0:2])
            totb_ps = ps_m.tile([128, 512], F32, tag="m", name="totb_ps")
            nc.tensor.matmul(totb_ps[:, 0:2], onesr128, tot_sb,
                             start=True, stop=True)
            muh = tact.tile([128, 1], F32, name="muh")
            nc.vector.tensor_scalar_mul(muh, totb_ps[:, 0:1], 1.0 / (512.0 * 512.0))
            e2h = tact.tile([128, 1], F32, name="e2h")
            nc.vector.tensor_scalar_mul(
                e2h, totb_ps[:, 1:2], 1.0 / (512.0 * 512.0 * 512.0)
            )
            musqh = tact.tile([128, 1], F32, name="musqh")
            nc.vector.tensor_mul(musqh, muh, muh)
            nc.vector.tensor_sub(e2h, e2h, musqh)
            sdh = tact.tile([128, 1], F32, name="sdh")
            nc.scalar.activation(out=sdh, in_=e2h, func=AF.Sqrt, bias=epsc)
            rstdh = tact.tile([128, 1], F32, name="rstdh")
            nc.vector.reciprocal(rstdh, sdh)
            pn4 = tact.tile([128, 4], F32, name="pn4")
            nc.vector.tensor_scalar(
                out=pn4, in0=pool4, scalar1=1.0 / 512.0, scalar2=muh,
                op0=ALU.mult, op1=ALU.subtract,
            )
            nc.vector.tensor_scalar_mul(pn4, pn4, rstdh)
            nc.vector.tensor_mul(pn4, pn4, h_g4_t)
            nc.vector.tensor_add(pn4, pn4, h_b4_t)
            y_ps = ps_m.tile([128, 512], F32, tag="m", name="y_ps")
            for k in range(4):
                nc.tensor.matmul(
                    y_ps[0:2, 0:1], h_w4_t[:, 2 * k : 2 * k + 2],
                    pn4[:, k : k + 1],
                    start=(k == 0), stop=(k == 3),
                )
            yo = tact.tile([2, 1], F32, name="yo")
            nc.vector.tensor_add(yo, y_ps[0:2, 0:1], h_b2_t)
            sdma(out=t["y_out"][:, :], in_=yo)


# --------------------------------------------------------------------------
# host glue
# --------------------------------------------------------------------------
def _col4(v):
    return np.ascontiguousarray(v.reshape(4, 128).T.astype(np.float32))


def _e_sel():
    e = np.zeros((8, 4, 128), dtype=np.float32)
    for i in range(4):
        e[2 * i, i, 0:64] = 1.0
        e[2 * i + 1, i, 64:128] = 1.0
    return np.ascontiguousarray(e)


def _host_flash_weights(I):
    """q2cT (centered+gained, f64) -> bf16 [32, 512]."""
    lat = I["latents"].astype(np.float64)
    mu = lat.mean(-1, keepdims=True)
    var = ((lat - mu) ** 2).mean(-1, keepdims=True)
    xq = (lat - mu) / np.sqrt(var + EPS) * I["c_ln_g"] + I["c_ln_b"]
    q = xq @ I["c_wq"].astype(np.float64)          # [512, 64]
    q2 = I["c_wk"].astype(np.float64) @ q.T        # [29, 512]
    q2g = q2 * I["ctx_ln_g"].astype(np.float64)[:, None]
    q2c = q2g - q2g.mean(0, keepdims=True)
    q2cT = np.zeros((2 * XF, LD), dtype=BF)
    q2cT[0:29, :] = q2c.astype(BF)
    q2cT[32:61, :] = q2c.astype(BF)
    return np.ascontiguousarray(q2cT)


def _prep_maps(inputs):
    I = {k: np.asarray(v, np.float32) for k, v in inputs.items()}
    enc = _fourier_pos()  # (26, T_FULL) f64
    q2cT = _host_flash_weights(I)

    def ffw1(w):
        return np.ascontiguousarray(
            w.reshape(4, 128, 16, 128).transpose(1, 2, 0, 3).astype(BF))

    def ffw2(w):
        return np.ascontiguousarray(
            w.reshape(16, 128, 4, 128).transpose(1, 0, 2, 3).astype(BF))

    def proj_w(w):
        return np.ascontiguousarray(
            w.reshape(4, 128, 4, 128).transpose(1, 2, 0, 3).astype(BF))

    shared = {
        "q2cT": q2cT,
        "c_wo_b": I["c_wo"].astype(BF),
        "c_bo4": _col4(I["c_bo"]),
        "w1c": ffw1(I["cf_w1"]),
        "cf_b1_16": np.ascontiguousarray(I["cf_b1"].reshape(16, 128).T),
        "w2c": ffw2(I["cf_w2"]),
        "cf_b2_4": _col4(I["cf_b2"]),
        "l_g4": _col4(I["l_ln_g"]),
        "l_b4": _col4(I["l_ln_b"]),
        "wq_l": proj_w(I["l_wq"]),
        "wk_l": proj_w(I["l_wk"]),
        "wv_l": np.ascontiguousarray(
            I["l_wv"].reshape(4, 128, 512).transpose(1, 0, 2).astype(BF)),
        "wo_l": proj_w(I["l_wo"]),
        "l_bo4": _col4(I["l_bo"]),
        "w1l": ffw1(I["lf_w1"]),
        "lf_b1_16": np.ascontiguousarray(I["lf_b1"].reshape(16, 128).T),
        "w2l": ffw2(I["lf_w2"]),
        "lf_b2_4": _col4(I["lf_b2"]),
        "h_g4": _col4(I["h_ln_g"]),
        "h_b4": _col4(I["h_ln_b"]),
        "h_w4": np.ascontiguousarray(
            I["h_w"].reshape(4, 128, 2).transpose(1, 0, 2).reshape(128, 8)
        ),
        "h_b2": I["h_b"][:, None],
        "e_sel": _e_sel(),
    }
    shared = {
        k: (np.ascontiguousarray(v, dtype=np.float32)
            if v.dtype != BF else v)
        for k, v in shared.items()
    }

    ctx_g = I["ctx_ln_g"].astype(np.float64)
    ctx_b = I["ctx_ln_b"].astype(np.float64)
    c_wv = I["c_wv"].astype(np.float64)

    maps = []
    for core in range(8):
        b, h = core // 2, core % 2
        data = I["data"][b].reshape(3, T_FULL)[:, h * T : (h + 1) * T]
        x = np.concatenate([data.astype(np.float64),
                            enc[:, h * T : (h + 1) * T]], 0)  # [29, T] f64
        mu = x.mean(0, keepdims=True)
        sd = np.sqrt(((x - mu) ** 2).mean(0, keepdims=True) + EPS)
        xn = (x - mu) / sd                                    # [29, T]
        xaT = np.zeros((2 * XF, NPAIR, 128), dtype=BF)
        xn3 = xn.reshape(29, NCHUNK, 128)
        xaT[0:29] = xn3[:, 0::2, :].astype(BF)
        xaT[32:61] = xn3[:, 1::2, :].astype(BF)
        # exact V with LN gain+bias, plus softmax-denominator ones column
        ln_x = xn * ctx_g[:, None] + ctx_b[:, None]
        v = (ln_x.T @ c_wv)                                   # [T, 64]
        v_all = np.ones((128, NCHUNK, 65), dtype=BF)
        v_all[:, :, 0:64] = v.reshape(NCHUNK, 128, 64).transpose(1, 0, 2).astype(BF)
        m = dict(shared)
        m["xaT"] = np.ascontiguousarray(xaT)
        m["v_all"] = np.ascontiguousarray(v_all)
        maps.append(m)
    return maps


def _get_nc():
    if "nc" not in _CACHE:
        _CACHE["nc"] = _build()
    return _CACHE["nc"]


def run_cores(inputs, **kw):
    nc = _get_nc()
    maps = _prep_maps(inputs)
    return run_bass_kernel_spmd(nc, maps, list(range(8)), **kw)


def kernel(**inputs) -> np.ndarray:
    res = run_cores(inputs)
    out = np.zeros((4, NC_CLS), np.float32)
    for b in range(4):
        out[b] = res.results[2 * b]["y"][:, 0]
    return out
